# revision 58
# baseline (speedup 1.0000x reference)
"""VQ-codebook encoding layer kernel for Trainium2 (8 NeuronCores).

Math (per batch row n):
    smooth[t,k] = scale[k] * (||x_t||^2 - 2<x_t, c_k> + ||c_k||^2)
    A = softmax_k(smooth)
    E[k,d] = sum_t A[t,k] * x[t,d]  -  (sum_t A[t,k]) * c[k,d]

Sharding: data-parallel over N across 8 cores (8 rows each), codebook +
scale replicated. No collectives needed (forward only).

v8 design notes (~83us HW; v2 baseline 118-137us was Vector-bound):
  - The ||x||^2 * scale_k term is folded into the cross-term PSUM via a
    second matmul per tile: qn[t,k] = xT^T @ W + xsqT^T @ Wsq, where
    W[d,k] = -2 scale_k c[k,d] and Wsq[d,k] = scale_k.  xsqT = xT*xT is
    one 2x-mode DVE multiply per half-row.  This deletes the square,
    the 4-op fold tree, the sqx*scale outer product and the qn+=vv pass
    (~2.7us/unit of vector-engine time) for ~0.5us/unit of PE time
    (128-col LDWEIGHTS pipeline behind the matmul stream; measured Q/MM2
    pitch 25ns, T/E pitch 56ns = stream-port floor, warm clock).
  - x loads are per-partition-contiguous casting SWDGE DMAs (~320 GB/s
    read-side vs the ~358 HBM/NC limit); codebook transpose and the PE
    identity come from DRAM (host-prepared) so no strided 4B DMA and no
    gpsimd-queue work delays the first load/transpose.
  - psum->sbuf xT copies: ACT bf16 copy (1x, ~2us), every 3rd unit on
    DVE as an int32-reinterpret copy (~1.2us, bit-exact; ACT's fp32
    path flushes denormal bit patterns so the u32 trick is DVE-only).
  - sum_t A rides a ones-weights matmul into a step-0 PSUM output AP
    (16 writes/slot accumulate via has_written); start=True clears
    has_written for the WHOLE PSUM bank (measured), so the first
    E-matmul carries the row's only start and everything else in the
    bank accumulates with start=False.
  - softmax tail: exp on ACT, sum_k + reciprocal_approx_fast on DVE,
    A = u8*rinv on GpSimd; 5-stage software pipeline (T -> CP -> XSQ ->
    Q/MM2 -> exp -> reduce -> normalize -> E) with lag-2/lag-4 spacing.
  - beta_k = scale_k*||c_k||^2 <= 2e-4 dropped (within bf16 noise).
"""

import numpy as np

import concourse.bass as bass
import concourse.bacc as bacc
import concourse.tile as tile
from concourse import mybir
from concourse import bass_utils
from concourse.masks import make_identity

N, T, K, D = 64, 4096, 32, 128
NCORES = 8
NP = N // NCORES          # rows per core
P = 128                   # partitions / token tile size
NTILES = T // P           # 32 token tiles per row
HT = NTILES // 2          # 16 token tiles per half-row unit

FP32 = mybir.dt.float32
BF16 = mybir.dt.bfloat16
U32 = mybir.dt.uint32

SUMA_STEP0 = True
CP_U32 = False

DBG = None  # debug-dump hooks (see debug_kernel.py)


def _build_bass():
    nc = bacc.Bacc("TRN2", target_bir_lowering=False, num_swdge_queues=4)
    x = nc.dram_tensor("x", (NP, T, D), FP32, kind="ExternalInput")
    cw = nc.dram_tensor("codewords", (K, D), FP32, kind="ExternalInput")
    cwT = nc.dram_tensor("codewordsT", (D, K), FP32, kind="ExternalInput")
    idm = nc.dram_tensor("identbf", (P, P // 2), FP32, kind="ExternalInput")
    sc = nc.dram_tensor("scale", (K,), FP32, kind="ExternalInput")
    out = nc.dram_tensor("out", (NP, K, D), FP32, kind="ExternalOutput")

    with tile.TileContext(nc) as tc:
        _kernel_body(tc, out[:], x[:], cw[:], cwT[:], idm[:], sc[:])
    nc.compile()
    return nc


def _kernel_body(tc, out, x, cw, cwT, idm, sc):
    nc = tc.nc
    MULT = mybir.AluOpType.mult
    ADD = mybir.AluOpType.add
    AXX = mybir.AxisListType.X
    EXP = mybir.ActivationFunctionType.Exp

    with (
        tc.tile_pool(name="consts", bufs=1) as consts,
        tc.tile_pool(name="xload", bufs=6) as xload,
        tc.tile_pool(name="xtp", bufs=5) as xtp,
        tc.tile_pool(name="xsqp", bufs=4) as xsqp,
        tc.tile_pool(name="sqp", bufs=2) as sqp,
        tc.tile_pool(name="soft", bufs=4) as soft,
        tc.tile_pool(name="outp", bufs=2) as outp,
        tc.tile_pool(name="ptr", bufs=2, space="PSUM") as ptr,
        tc.tile_pool(name="pq", bufs=2, space="PSUM") as pq,
        tc.tile_pool(name="pe", bufs=2, space="PSUM") as pe_pool,
    ):
        # ---------------- setup (once) ----------------
        # identity arrives as fp32-encoded bf16 pairs via HWDGE (keeps the
        # gpsimd queue free for the x-load emissions at kernel start)
        ident = consts.tile([P, P], BF16)         # PE-transpose identity
        nc.sync.dma_start(ident[:].bitcast(FP32), idm)

        c_sb = consts.tile([K, D], FP32)          # c[k,d]
        nc.sync.dma_start(c_sb[:], cw)
        cT_sb = consts.tile([D, K], FP32)         # c^T[d,k] (host-transposed)
        nc.sync.dma_start(cT_sb[:], cwT)
        scale_bc = consts.tile([P, K], FP32)      # scale[k] on 128 partitions
        nc.sync.dma_start(scale_bc[:], sc[None, :].to_broadcast((P, K)))

        # W[d,k] = -2 * scale_k * c^T  (bf16)
        W = consts.tile([D, K], BF16)
        nc.vector.scalar_tensor_tensor(
            out=W[:], in0=cT_sb[:], scalar=-2.0, in1=scale_bc[0:D, :],
            op0=MULT, op1=MULT,
        )
        # Wsq[d,k] = scale_k  (bf16) -- MM2 turns sum_d xsqT[d,t]*Wsq[d,k]
        # into scale_k * ||x_t||^2 accumulated straight into the qn psum.
        Wsq = consts.tile([D, K], BF16)
        nc.scalar.copy(Wsq[:], scale_bc[0:D, :])

        ones_col = consts.tile([P, 1], BF16)      # sum_t A weights
        nc.vector.memset(ones_col[:], 1.0)
        ones11 = consts.tile([1, 1], BF16)        # mini-transpose moving op
        nc.vector.memset(ones11[:], 1.0)
        c_neg = consts.tile([K, D], FP32)         # -c for the final fixup
        nc.scalar.mul(c_neg[:], c_sb[:], -1.0)

        # ---------------- per-unit state ----------------
        units = [(n, h) for n in range(NP) for h in range(2)]
        xbfs = {}     # row  -> xbf [P, NTILES, D] bf16
        ptrs = {}     # unit -> psum transpose tile [D, HT, P] bf16
        xTs = {}      # unit -> xT sbuf [D, HT, P] bf16
        xsqs = {}     # unit -> xsqT sbuf [D, HT, P] bf16
        qns = {}      # unit -> qn psum [P, HT, K] fp32
        u8s = {}      # unit -> u8 [P, HT, K] bf16
        rinvs = {}    # unit -> rinv [P, HT] fp32
        ans = {}      # unit -> an [P, HT, K] bf16
        pes = {}      # row  -> psum E tile [K, 192] fp32

        def load_row(n, nsplit=1):
            xbf = xload.tile([P, NTILES, D], BF16)
            step = NTILES // nsplit
            for g in range(nsplit):
                nc.gpsimd.dma_start(
                    out=xbf[:, g * step : (g + 1) * step, :],
                    in_=x[n].rearrange("(p i) d -> p i d", p=P)[
                        :, g * step : (g + 1) * step, :
                    ],
                )
            xbfs[n] = xbf


        def phase_T(u):
            # PE: transpose the unit's 16 token tiles into one psum tile
            n, h = u
            xbf = xbfs[n]
            pt = ptr.tile([D, HT, P], BF16)
            for jj in range(HT):
                nc.tensor.transpose(
                    pt[:, jj, :], xbf[:, h * HT + jj, :], ident[:]
                )
            ptrs[u] = pt

        def phase_CP(u, i):
            # psum -> sbuf copy of the unit's transposes.  ACT copy runs at
            # 1x (~2us); every 3rd unit goes to DVE as an int32-reinterpret
            # copy instead (bit-exact on DVE, ~1.2us; ACT's fp32 path
            # flushes denormal bit patterns so the u32 trick is DVE-only).
            pt = ptrs.pop(u)
            xT = xtp.tile([D, HT, P], BF16)
            # last two units force the faster DVE copy (shortens the serial
            # drain chain at kernel end)
            if i % 3 == 2 or i >= NU - 2:
                nc.vector.tensor_copy(xT[:].bitcast(U32), pt[:].bitcast(U32))
            else:
                nc.scalar.copy(xT[:], pt[:])
            xTs[u] = xT
            if DBG and u == (0, 0):
                nc.gpsimd.dma_start(out=DBG["xT"], in_=xT[:])

        def phase_XSQ(u):
            # DVE: xsqT = xT * xT (2x-mode bf16 multiply)
            xT = xTs[u]
            xsq = xsqp.tile([D, HT, P], BF16)
            nc.vector.tensor_mul(xsq[:], xT[:], xT[:])
            xsqs[u] = xsq

        def phase_Q(u):
            # PE: qn[t,k] = -2 scale_k <x_t, c_k> + scale_k ||x_t||^2
            xT = xTs.pop(u)
            xsq = xsqs.pop(u)
            qn = pq.tile([P, HT, K], FP32)
            for jj in range(HT):
                nc.tensor.matmul(
                    qn[:, jj, :], lhsT=xT[:, jj, :], rhs=W[:],
                    start=(jj == 0), stop=False, skip_group_check=True,
                )
            for jj in range(HT):
                nc.tensor.matmul(
                    qn[:, jj, :], lhsT=xsq[:, jj, :], rhs=Wsq[:],
                    start=False, stop=(jj == HT - 1), skip_group_check=True,
                )
            qns[u] = qn

        def phase_EX(u):
            # ACT: u8 = exp(qn)
            qn = qns.pop(u)
            u8 = soft.tile([P, HT, K], BF16, tag="u8")
            nc.scalar.activation(u8[:], qn[:], EXP)
            u8s[u] = u8
            if DBG and u == (0, 0):
                nc.gpsimd.dma_start(out=DBG["u8"], in_=u8[:])

        def phase_RS(u):
            # DVE: s = sum_k u8 ; rinv = 1/s
            u8 = u8s[u]
            s = sqp.tile([P, HT], FP32, tag="s")
            nc.vector.reduce_sum(s[:], u8[:], AXX)
            rinv = sqp.tile([P, HT], FP32, tag="rinv")
            nc.vector.reciprocal_approx_fast(rinv[:], s[:])
            rinvs[u] = rinv

        def phase_AN(u):
            # GPSIMD: an = u8 * rinv (last row on DVE: faster, shortens the
            # end-of-kernel drain chain)
            u8 = u8s.pop(u)
            rinv = rinvs.pop(u)
            an = soft.tile([P, HT, K], BF16, tag="an")
            eng = nc.vector if u[0] == NP - 1 else nc.gpsimd
            eng.tensor_mul(
                an[:], u8[:], rinv[:, :, None].to_broadcast((P, HT, K))
            )
            ans[u] = an
            if DBG and u == (0, 0):
                nc.gpsimd.dma_start(out=DBG["an"], in_=an[:])

        def phase_E(u):
            n, h = u
            xbf = xbfs[n]
            an = ans.pop(u)
            if h == 0:
                pes[n] = pe_pool.tile([K, 192], FP32, name="psum_E", tag="pE")
            pe = pes[n]
            for jj in range(HT):
                nc.tensor.matmul(
                    pe[:, 0:D], lhsT=an[:, jj, :], rhs=xbf[:, h * HT + jj, :],
                    start=(h == 0 and jj == 0), stop=(h == 1 and jj == HT - 1),
                    skip_group_check=True,
                )
            # sum_t A[t,k] -> pe[0, 128+k], accumulated over both halves.
            # start=False always: a start=True here would clear the whole
            # PSUM bank's has_written bits and wipe the E accumulation
            # (observed on HW); the first E matmul's start=True clears the
            # bank once per row, covering this region too.
            if SUMA_STEP0:
                sa_out = pe[0:1, 128:160][:, None, :].to_broadcast((1, HT, K))
                nc.tensor.matmul(
                    sa_out, lhsT=ones_col[:], rhs=an[:],
                    start=False, stop=(h == 1), skip_group_check=True,
                )
            else:
                for jj in range(HT):
                    nc.tensor.matmul(
                        pe[0:1, 128:160], lhsT=ones_col[:], rhs=an[:, jj, :],
                        start=False, stop=(h == 1 and jj == HT - 1),
                        skip_group_check=True,
                    )
        sa_sbs = {}   # row -> [1, K] bf16 sum_t A staging

        def finish_a(n):
            # DVE: [1,K] row of sums -> sbuf (emitted at end of DVE queue)
            pe = pes[n]
            if DBG and n == 0:
                scr = outp.tile([K, D], FP32, tag="dbgE")
                nc.vector.tensor_copy(scr[:], pe[:, 0:D])
                nc.gpsimd.dma_start(out=DBG["Eraw"], in_=scr[:])
                scr2 = outp.tile([1, 64], FP32, tag="dbgSA")
                nc.vector.tensor_copy(scr2[:, 0:32], pe[0:1, 128:160])
                nc.gpsimd.dma_start(out=DBG["sumA"], in_=scr2[:, 0:32])
            sa_sb = outp.tile([1, K], BF16, tag="sa")
            nc.vector.tensor_copy(sa_sb[:], pe[0:1, 128:160])
            sa_sbs[n] = sa_sb

        def finish_b(n):
            # PE: [1,K] -> [K,1] column via mini-matmul (next iteration)
            pe = pes[n]
            sa_sb = sa_sbs.pop(n)
            nc.tensor.matmul(
                pe[:, 160:161], lhsT=sa_sb[:], rhs=ones11[:],
                start=True, stop=True, skip_group_check=True,
            )

        def finish_c(n):
            # DVE + DMA: E[k,d] = raw - sumA_k * c[k,d]; store
            pe = pes.pop(n)
            xbfs.pop(n)
            e_sb = outp.tile([K, D], FP32, tag="e")
            nc.vector.scalar_tensor_tensor(
                out=e_sb[:], in0=c_neg[:], scalar=pe[:, 160:161],
                in1=pe[:, 0:D], op0=MULT, op1=ADD,
            )
            nc.sync.dma_start(out[n], e_sb[:])

        # ---------------- software-pipelined main loop ----------------
        # per iteration i (unit u_i):
        #   PE : [finish_b] T(i)   Q+MM2(i-2)   E(i-4)
        #   ACT: EX(i-3)           CP(i)
        #   DVE: XSQ(i-1)  RS(i-3) [finish_c]   [finish_a]
        #   GPS: [loads]           AN(i-3)
        NU = len(units)
        load_row(0, nsplit=8)
        load_row(1, nsplit=2)
        fin_next = []   # rows whose finish_b/finish_c run this iteration

        def iteration(i, u):
            nonlocal fin_next
            if u is not None and u[1] == 0 and u[0] + 2 < NP:
                load_row(u[0] + 2)
            # PE queue: DMA-independent work first — the queue is in-order,
            # so a data-starved T(i) at the head would block ready Q/E work
            # behind it during the fill phase
            for nf in fin_next:
                finish_b(nf)
            if i >= 2 and i - 2 < NU:
                phase_Q(units[i - 2])
            if i >= 4 and i - 4 < NU:
                phase_E(units[i - 4])
            if u is not None:
                phase_T(u)
            # ACT queue
            if i >= 3 and i - 3 < NU:
                phase_EX(units[i - 3])
            if u is not None:
                phase_CP(u, i)
            # DVE queue (XSQ first: its dep is a full iteration old)
            if i >= 1 and i - 1 < NU:
                phase_XSQ(units[i - 1])
            if i >= 3 and i - 3 < NU:
                phase_RS(units[i - 3])
            for nf in fin_next:
                finish_c(nf)
            fin_next = []
            if i >= 4 and i - 4 < NU and units[i - 4][1] == 1:
                finish_a(units[i - 4][0])
                fin_next.append(units[i - 4][0])
            # GPSIMD queue
            if i >= 3 and i - 3 < NU:
                phase_AN(units[i - 3])

        for i in range(NU + 4):
            iteration(i, units[i] if i < NU else None)
        for nf in fin_next:
            finish_b(nf)
            finish_c(nf)


_NC_CACHE = None


def _identbf():
    # 128x128 bf16 identity, packed as fp32 words for a cast-free HWDGE load
    u = np.zeros((P, P), np.uint16)
    u[np.arange(P), np.arange(P)] = 0x3F80  # bf16 1.0
    return np.ascontiguousarray(u.view(np.float32))


def _get_nc():
    global _NC_CACHE
    if _NC_CACHE is None:
        _NC_CACHE = _build_bass()
    return _NC_CACHE


def kernel(**inputs):
    x = np.ascontiguousarray(np.asarray(inputs["x"], dtype=np.float32))
    cw = np.ascontiguousarray(np.asarray(inputs["codewords"], dtype=np.float32))
    sc = np.ascontiguousarray(np.asarray(inputs["scale"], dtype=np.float32))

    nc = _get_nc()
    cwT = np.ascontiguousarray(cw.T)
    in_maps = [
        {"x": x[i * NP : (i + 1) * NP], "codewords": cw, "codewordsT": cwT,
         "identbf": _identbf(), "scale": sc}
        for i in range(NCORES)
    ]
    res = bass_utils.run_bass_kernel_spmd(nc, in_maps, core_ids=list(range(NCORES)))
    return np.concatenate([r["out"] for r in res.results], axis=0)


if __name__ == "__main__":
    rng = np.random.default_rng(0)
    ins = {
        "x": rng.standard_normal((N, T, D), dtype=np.float32),
        "codewords": rng.uniform(-0.01, 0.01, (K, D)).astype(np.float32),
        "scale": rng.uniform(-0.01, 0.01, (K,)).astype(np.float32),
    }
    out = kernel(**ins)
    print(out.shape, out.dtype)

    # numpy reference check
    xx = ins["x"]; c = ins["codewords"]; s = ins["scale"]
    sqx = (xx * xx).sum(-1, keepdims=True)
    cross = xx @ c.T
    sqc = (c * c).sum(-1)
    sm = s * (sqx - 2 * cross + sqc)
    sm -= sm.max(-1, keepdims=True)
    A = np.exp(sm); A /= A.sum(-1, keepdims=True)
    E = np.einsum("ntk,ntd->nkd", A, xx) - A.sum(1)[:, :, None] * c
    err = np.abs(out - E).max() / np.abs(E).max()
    print("rel err vs numpy:", err)


# revision 59
# speedup vs baseline: 1.0316x; 1.0316x over previous
"""VQ-codebook encoding layer kernel for Trainium2 (8 NeuronCores).

Math (per batch row n):
    smooth[t,k] = scale[k] * (||x_t||^2 - 2<x_t, c_k> + ||c_k||^2)
    A = softmax_k(smooth)
    E[k,d] = sum_t A[t,k] * x[t,d]  -  (sum_t A[t,k]) * c[k,d]

Sharding: data-parallel over N across 8 cores (8 rows each), codebook +
scale replicated. No collectives needed (forward only).

v8 design notes (~83us HW; v2 baseline 118-137us was Vector-bound):
  - The ||x||^2 * scale_k term is folded into the cross-term PSUM via a
    second matmul per tile: qn[t,k] = xT^T @ W + xsqT^T @ Wsq, where
    W[d,k] = -2 scale_k c[k,d] and Wsq[d,k] = scale_k.  xsqT = xT*xT is
    one 2x-mode DVE multiply per half-row.  This deletes the square,
    the 4-op fold tree, the sqx*scale outer product and the qn+=vv pass
    (~2.7us/unit of vector-engine time) for ~0.5us/unit of PE time
    (128-col LDWEIGHTS pipeline behind the matmul stream; measured Q/MM2
    pitch 25ns, T/E pitch 56ns = stream-port floor, warm clock).
  - x loads are per-partition-contiguous casting SWDGE DMAs (~320 GB/s
    read-side vs the ~358 HBM/NC limit); codebook transpose and the PE
    identity come from DRAM (host-prepared) so no strided 4B DMA and no
    gpsimd-queue work delays the first load/transpose.
  - psum->sbuf xT copies: ACT bf16 copy (1x, ~2us), every 3rd unit on
    DVE as an int32-reinterpret copy (~1.2us, bit-exact; ACT's fp32
    path flushes denormal bit patterns so the u32 trick is DVE-only).
  - sum_t A rides a ones-weights matmul into a step-0 PSUM output AP
    (16 writes/slot accumulate via has_written); start=True clears
    has_written for the WHOLE PSUM bank (measured), so the first
    E-matmul carries the row's only start and everything else in the
    bank accumulates with start=False.
  - softmax tail: exp on ACT, sum_k + reciprocal_approx_fast on DVE,
    A = u8*rinv on GpSimd; 5-stage software pipeline (T -> CP -> XSQ ->
    Q/MM2 -> exp -> reduce -> normalize -> E) with lag-2/lag-4 spacing.
  - beta_k = scale_k*||c_k||^2 <= 2e-4 dropped (within bf16 noise).
"""

import numpy as np

import concourse.bass as bass
import concourse.bacc as bacc
import concourse.tile as tile
from concourse import mybir
from concourse import bass_utils
from concourse.masks import make_identity

N, T, K, D = 64, 4096, 32, 128
NCORES = 8
NP = N // NCORES          # rows per core
P = 128                   # partitions / token tile size
NTILES = T // P           # 32 token tiles per row
HT = NTILES // 2          # 16 token tiles per half-row unit

FP32 = mybir.dt.float32
BF16 = mybir.dt.bfloat16
U32 = mybir.dt.uint32

SUMA_STEP0 = True
CP_U32 = False

DBG = None  # debug-dump hooks (see debug_kernel.py)


def _build_bass():
    nc = bacc.Bacc("TRN2", target_bir_lowering=False, num_swdge_queues=4)
    x = nc.dram_tensor("x", (NP, T, D), FP32, kind="ExternalInput")
    cw = nc.dram_tensor("codewords", (K, D), FP32, kind="ExternalInput")
    cwT = nc.dram_tensor("codewordsT", (D, K), FP32, kind="ExternalInput")
    idm = nc.dram_tensor("identbf", (P, P // 2), FP32, kind="ExternalInput")
    sc = nc.dram_tensor("scale", (K,), FP32, kind="ExternalInput")
    out = nc.dram_tensor("out", (NP, K, D), FP32, kind="ExternalOutput")

    with tile.TileContext(nc) as tc:
        _kernel_body(tc, out[:], x[:], cw[:], cwT[:], idm[:], sc[:])
    nc.compile()
    return nc


def _kernel_body(tc, out, x, cw, cwT, idm, sc):
    nc = tc.nc
    MULT = mybir.AluOpType.mult
    ADD = mybir.AluOpType.add
    AXX = mybir.AxisListType.X
    EXP = mybir.ActivationFunctionType.Exp

    with (
        tc.tile_pool(name="consts", bufs=1) as consts,
        tc.tile_pool(name="xload", bufs=6) as xload,
        tc.tile_pool(name="xtp", bufs=5) as xtp,
        tc.tile_pool(name="xsqp", bufs=4) as xsqp,
        tc.tile_pool(name="sqp", bufs=2) as sqp,
        tc.tile_pool(name="soft", bufs=4) as soft,
        tc.tile_pool(name="outp", bufs=2) as outp,
        tc.tile_pool(name="ptr", bufs=2, space="PSUM") as ptr,
        tc.tile_pool(name="pq", bufs=2, space="PSUM") as pq,
        tc.tile_pool(name="pe", bufs=2, space="PSUM") as pe_pool,
    ):
        # ---------------- setup (once) ----------------
        # identity arrives as fp32-encoded bf16 pairs via HWDGE (keeps the
        # gpsimd queue free for the x-load emissions at kernel start)
        ident = consts.tile([P, P], BF16)         # PE-transpose identity
        nc.sync.dma_start(ident[:].bitcast(FP32), idm)

        c_sb = consts.tile([K, D], FP32)          # c[k,d]
        nc.sync.dma_start(c_sb[:], cw)
        cT_sb = consts.tile([D, K], FP32)         # c^T[d,k] (host-transposed)
        nc.sync.dma_start(cT_sb[:], cwT)
        scale_bc = consts.tile([P, K], FP32)      # scale[k] on 128 partitions
        nc.sync.dma_start(scale_bc[:], sc[None, :].to_broadcast((P, K)))

        # W[d,k] = -2 * scale_k * c^T  (bf16)
        W = consts.tile([D, K], BF16)
        nc.vector.scalar_tensor_tensor(
            out=W[:], in0=cT_sb[:], scalar=-2.0, in1=scale_bc[0:D, :],
            op0=MULT, op1=MULT,
        )
        # Wsq[d,k] = scale_k  (bf16) -- MM2 turns sum_d xsqT[d,t]*Wsq[d,k]
        # into scale_k * ||x_t||^2 accumulated straight into the qn psum.
        Wsq = consts.tile([D, K], BF16)
        nc.scalar.copy(Wsq[:], scale_bc[0:D, :])

        ones_col = consts.tile([P, 1], BF16)      # sum_t A weights
        nc.vector.memset(ones_col[:], 1.0)
        ones11 = consts.tile([1, 1], BF16)        # mini-transpose moving op
        nc.vector.memset(ones11[:], 1.0)
        c_neg = consts.tile([K, D], FP32)         # -c for the final fixup
        nc.scalar.mul(c_neg[:], c_sb[:], -1.0)

        # ---------------- per-unit state ----------------
        units = [(n, h) for n in range(NP) for h in range(2)]
        xbfs = {}     # row  -> xbf [P, NTILES, D] bf16
        ptrs = {}     # unit -> psum transpose tile [D, HT, P] bf16
        xTs = {}      # unit -> xT sbuf [D, HT, P] bf16
        xsqs = {}     # unit -> xsqT sbuf [D, HT, P] bf16
        qns = {}      # unit -> qn psum [P, HT, K] fp32
        u8s = {}      # unit -> u8 [P, HT, K] bf16
        rinvs = {}    # unit -> rinv [P, HT] fp32
        ans = {}      # unit -> an [P, HT, K] bf16
        pes = {}      # row  -> psum E tile [K, 192] fp32

        def load_row(n, nsplit=1):
            xbf = xload.tile([P, NTILES, D], BF16)
            step = NTILES // nsplit
            for g in range(nsplit):
                nc.gpsimd.dma_start(
                    out=xbf[:, g * step : (g + 1) * step, :],
                    in_=x[n].rearrange("(p i) d -> p i d", p=P)[
                        :, g * step : (g + 1) * step, :
                    ],
                )
            xbfs[n] = xbf


        def phase_T(u):
            # PE: transpose the unit's 16 token tiles into one psum tile
            n, h = u
            xbf = xbfs[n]
            pt = ptr.tile([D, HT, P], BF16)
            for jj in range(HT):
                nc.tensor.transpose(
                    pt[:, jj, :], xbf[:, h * HT + jj, :], ident[:]
                )
            ptrs[u] = pt

        def phase_CP(u, i):
            # psum -> sbuf copy of the unit's transposes.  ACT copy runs at
            # 1x (~2us); every 3rd unit goes to DVE as an int32-reinterpret
            # copy instead (bit-exact on DVE, ~1.2us; ACT's fp32 path
            # flushes denormal bit patterns so the u32 trick is DVE-only).
            pt = ptrs.pop(u)
            xT = xtp.tile([D, HT, P], BF16)
            # last two units force the faster DVE copy (shortens the serial
            # drain chain at kernel end)
            if i % 3 == 2 or i >= NU - 2:
                nc.vector.tensor_copy(xT[:].bitcast(U32), pt[:].bitcast(U32))
            else:
                nc.scalar.copy(xT[:], pt[:])
            xTs[u] = xT
            if DBG and u == (0, 0):
                nc.gpsimd.dma_start(out=DBG["xT"], in_=xT[:])

        def phase_XSQ(u):
            # DVE: xsqT = xT * xT (2x-mode bf16 multiply)
            xT = xTs[u]
            xsq = xsqp.tile([D, HT, P], BF16)
            nc.vector.tensor_mul(xsq[:], xT[:], xT[:])
            xsqs[u] = xsq

        def phase_Q(u):
            # PE: qn[t,k] = -2 scale_k <x_t, c_k> + scale_k ||x_t||^2
            xT = xTs.pop(u)
            xsq = xsqs.pop(u)
            qn = pq.tile([P, HT, K], FP32)
            for jj in range(HT):
                nc.tensor.matmul(
                    qn[:, jj, :], lhsT=xT[:, jj, :], rhs=W[:],
                    start=(jj == 0), stop=False, skip_group_check=True,
                )
            for jj in range(HT):
                nc.tensor.matmul(
                    qn[:, jj, :], lhsT=xsq[:, jj, :], rhs=Wsq[:],
                    start=False, stop=(jj == HT - 1), skip_group_check=True,
                )
            qns[u] = qn

        def phase_EX(u):
            # ACT: u8 = exp(qn)
            qn = qns.pop(u)
            u8 = soft.tile([P, HT, K], BF16, tag="u8")
            nc.scalar.activation(u8[:], qn[:], EXP)
            u8s[u] = u8
            if DBG and u == (0, 0):
                nc.gpsimd.dma_start(out=DBG["u8"], in_=u8[:])

        def phase_RS(u):
            # DVE: s = sum_k u8 ; rinv = 1/s
            u8 = u8s[u]
            s = sqp.tile([P, HT], FP32, tag="s")
            nc.vector.reduce_sum(s[:], u8[:], AXX)
            rinv = sqp.tile([P, HT], FP32, tag="rinv")
            nc.vector.reciprocal_approx_fast(rinv[:], s[:])
            rinvs[u] = rinv

        def phase_AN(u):
            # GPSIMD: an = u8 * rinv (last row on DVE: faster, shortens the
            # end-of-kernel drain chain)
            u8 = u8s.pop(u)
            rinv = rinvs.pop(u)
            an = soft.tile([P, HT, K], BF16, tag="an")
            eng = nc.vector if u[0] == NP - 1 else nc.gpsimd
            eng.tensor_mul(
                an[:], u8[:], rinv[:, :, None].to_broadcast((P, HT, K))
            )
            ans[u] = an
            if DBG and u == (0, 0):
                nc.gpsimd.dma_start(out=DBG["an"], in_=an[:])

        def phase_E(u):
            n, h = u
            xbf = xbfs[n]
            an = ans.pop(u)
            if h == 0:
                pes[n] = pe_pool.tile([K, 192], FP32, name="psum_E", tag="pE")
            pe = pes[n]
            for jj in range(HT):
                nc.tensor.matmul(
                    pe[:, 0:D], lhsT=an[:, jj, :], rhs=xbf[:, h * HT + jj, :],
                    start=(h == 0 and jj == 0), stop=(h == 1 and jj == HT - 1),
                    skip_group_check=True,
                )
            # sum_t A[t,k] -> pe[0, 128+k], accumulated over both halves.
            # start=False always: a start=True here would clear the whole
            # PSUM bank's has_written bits and wipe the E accumulation
            # (observed on HW); the first E matmul's start=True clears the
            # bank once per row, covering this region too.
            if SUMA_STEP0:
                sa_out = pe[0:1, 128:160][:, None, :].to_broadcast((1, HT, K))
                nc.tensor.matmul(
                    sa_out, lhsT=ones_col[:], rhs=an[:],
                    start=False, stop=(h == 1), skip_group_check=True,
                )
            else:
                for jj in range(HT):
                    nc.tensor.matmul(
                        pe[0:1, 128:160], lhsT=ones_col[:], rhs=an[:, jj, :],
                        start=False, stop=(h == 1 and jj == HT - 1),
                        skip_group_check=True,
                    )
        sa_sbs = {}   # row -> [1, K] bf16 sum_t A staging

        def finish_a(n):
            # DVE: [1,K] row of sums -> sbuf (emitted at end of DVE queue)
            pe = pes[n]
            if DBG and n == 0:
                scr = outp.tile([K, D], FP32, tag="dbgE")
                nc.vector.tensor_copy(scr[:], pe[:, 0:D])
                nc.gpsimd.dma_start(out=DBG["Eraw"], in_=scr[:])
                scr2 = outp.tile([1, 64], FP32, tag="dbgSA")
                nc.vector.tensor_copy(scr2[:, 0:32], pe[0:1, 128:160])
                nc.gpsimd.dma_start(out=DBG["sumA"], in_=scr2[:, 0:32])
            sa_sb = outp.tile([1, K], BF16, tag="sa")
            nc.vector.tensor_copy(sa_sb[:], pe[0:1, 128:160])
            sa_sbs[n] = sa_sb

        def finish_b(n):
            # PE: [1,K] -> [K,1] column via mini-matmul (next iteration)
            pe = pes[n]
            sa_sb = sa_sbs.pop(n)
            nc.tensor.matmul(
                pe[:, 160:161], lhsT=sa_sb[:], rhs=ones11[:],
                start=True, stop=True, skip_group_check=True,
            )

        def finish_c(n):
            # DVE + DMA: E[k,d] = raw - sumA_k * c[k,d]; store
            pe = pes.pop(n)
            xbfs.pop(n)
            e_sb = outp.tile([K, D], FP32, tag="e")
            nc.vector.scalar_tensor_tensor(
                out=e_sb[:], in0=c_neg[:], scalar=pe[:, 160:161],
                in1=pe[:, 0:D], op0=MULT, op1=ADD,
            )
            nc.sync.dma_start(out[n], e_sb[:])

        # ---------------- software-pipelined main loop ----------------
        # per iteration i (unit u_i):
        #   PE : [finish_b] T(i)   Q+MM2(i-2)   E(i-4)
        #   ACT: EX(i-3)           CP(i)
        #   DVE: XSQ(i-1)  RS(i-3) [finish_c]   [finish_a]
        #   GPS: [loads]           AN(i-3)
        NU = len(units)
        load_row(0, nsplit=8)
        load_row(1, nsplit=2)
        fin_next = []   # rows whose finish_b/finish_c run this iteration

        def iteration(i, u):
            nonlocal fin_next
            if u is not None and u[1] == 0 and u[0] + 2 < NP:
                load_row(u[0] + 2)
            # PE queue
            for nf in fin_next:
                finish_b(nf)
            if u is not None:
                phase_T(u)
            if i >= 2 and i - 2 < NU:
                phase_Q(units[i - 2])
            if i >= 4 and i - 4 < NU:
                phase_E(units[i - 4])
            # ACT queue
            if i >= 3 and i - 3 < NU:
                phase_EX(units[i - 3])
            if u is not None:
                phase_CP(u, i)
            # DVE queue (XSQ first: its dep is a full iteration old)
            if i >= 1 and i - 1 < NU:
                phase_XSQ(units[i - 1])
            if i >= 3 and i - 3 < NU:
                phase_RS(units[i - 3])
            for nf in fin_next:
                finish_c(nf)
            fin_next = []
            if i >= 4 and i - 4 < NU and units[i - 4][1] == 1:
                finish_a(units[i - 4][0])
                fin_next.append(units[i - 4][0])
            # GPSIMD queue
            if i >= 3 and i - 3 < NU:
                phase_AN(units[i - 3])

        for i in range(NU + 4):
            iteration(i, units[i] if i < NU else None)
        for nf in fin_next:
            finish_b(nf)
            finish_c(nf)


_NC_CACHE = None


def _identbf():
    # 128x128 bf16 identity, packed as fp32 words for a cast-free HWDGE load
    u = np.zeros((P, P), np.uint16)
    u[np.arange(P), np.arange(P)] = 0x3F80  # bf16 1.0
    return np.ascontiguousarray(u.view(np.float32))


def _get_nc():
    global _NC_CACHE
    if _NC_CACHE is None:
        _NC_CACHE = _build_bass()
    return _NC_CACHE


def kernel(**inputs):
    x = np.ascontiguousarray(np.asarray(inputs["x"], dtype=np.float32))
    cw = np.ascontiguousarray(np.asarray(inputs["codewords"], dtype=np.float32))
    sc = np.ascontiguousarray(np.asarray(inputs["scale"], dtype=np.float32))

    nc = _get_nc()
    cwT = np.ascontiguousarray(cw.T)
    in_maps = [
        {"x": x[i * NP : (i + 1) * NP], "codewords": cw, "codewordsT": cwT,
         "identbf": _identbf(), "scale": sc}
        for i in range(NCORES)
    ]
    res = bass_utils.run_bass_kernel_spmd(nc, in_maps, core_ids=list(range(NCORES)))
    return np.concatenate([r["out"] for r in res.results], axis=0)


if __name__ == "__main__":
    rng = np.random.default_rng(0)
    ins = {
        "x": rng.standard_normal((N, T, D), dtype=np.float32),
        "codewords": rng.uniform(-0.01, 0.01, (K, D)).astype(np.float32),
        "scale": rng.uniform(-0.01, 0.01, (K,)).astype(np.float32),
    }
    out = kernel(**ins)
    print(out.shape, out.dtype)

    # numpy reference check
    xx = ins["x"]; c = ins["codewords"]; s = ins["scale"]
    sqx = (xx * xx).sum(-1, keepdims=True)
    cross = xx @ c.T
    sqc = (c * c).sum(-1)
    sm = s * (sqx - 2 * cross + sqc)
    sm -= sm.max(-1, keepdims=True)
    A = np.exp(sm); A /= A.sum(-1, keepdims=True)
    E = np.einsum("ntk,ntd->nkd", A, xx) - A.sum(1)[:, :, None] * c
    err = np.abs(out - E).max() / np.abs(E).max()
    print("rel err vs numpy:", err)
